# revision 33
# baseline (speedup 1.0000x reference)
import sys, os
sys.path.insert(0, '/opt/trn_rl_repo')
import numpy as np
import ml_dtypes

import concourse.bass as bass
import concourse.bacc as bacc
import concourse.tile as tile
from concourse import mybir, bass_utils

F32 = mybir.dt.float32
_f32r_lvl = int(os.environ.get("BNN_F32R", "0"))
F32R = mybir.dt.float32r if _f32r_lvl >= 1 else mybir.dt.float32
F32R2 = mybir.dt.float32r if _f32r_lvl >= 2 else mybir.dt.float32
BF16 = mybir.dt.bfloat16
ACTF = mybir.ActivationFunctionType
ALU = mybir.AluOpType
AX = mybir.AxisListType
NC, B, EPS = 8, 32, 1e-5
RG = [list(range(NC))]
_cache = {}

A1W = 7144                      # 4 images of padded 42x42 + slack
P2W = 16 * 484                  # stage-2 pooled, 22-padded per image
B2W = 24 + B * 484 + 24         # stage-3 input row width
N1 = 256.0 * 6400.0


def _build(dbg=False):
    nc = bacc.Bacc("TRN2", target_bir_lowering=False, debug=False, num_devices=NC)

    def din(name, shape, dt=F32):
        return nc.dram_tensor(name, list(shape), dt, kind="ExternalInput").ap()

    dbg_outs = {}
    def dout(name, shape, dt=F32):
        dbg_outs[name] = nc.dram_tensor(name, list(shape), dt, kind="ExternalOutput").ap()
        return dbg_outs[name]

    xim  = din("xim", [8, 128, 6400], F32R)
    w1b  = din("w1b", [36, 128], F32R)
    w2   = din("w2", [96, 192], F32R2)
    w3p  = din("w3p", [128, 768], BF16)
    wfc  = din("wfc", [128, 6400], BF16)
    wfco = din("wfco", [128, 24])
    fcob = din("fcob", [12, 1])
    ff4  = din("ff4", [128, 32])
    fb4  = din("fb4", [32, 128])
    ff2  = din("ff2", [128, 64])
    fb2  = din("fb2", [64, 128])
    out  = nc.dram_tensor("out", [12, B], F32, kind="ExternalOutput").ap()

    ENG3 = None  # set inside context

    with tile.TileContext(nc) as tc:
      with tc.tile_pool(name="pm", bufs=1) as pm, \
           tc.tile_pool(name="pd", bufs=1, space="DRAM") as pd:
        ENG3 = [nc.sync, nc.scalar, nc.gpsimd]

        def allred(tag, src, K):
            bi = pd.tile([128, K], F32, tag="ari" + tag, name="ari" + tag)
            bo = pd.tile([128 * NC, K], F32, tag="aro" + tag, name="aro" + tag)
            nc.sync.dma_start(bi[:], src)
            nc.gpsimd.collective_compute("AllGather", ALU.bypass, replica_groups=RG,
                                         ins=[bi.opt()], outs=[bo.opt()])
            gsb = pm.tile([128, K * NC], F32, tag="arg" + tag, name="arg" + tag)
            nc.sync.dma_start(gsb[:].rearrange("p (k r) -> p k r", r=NC),
                              bo[:].rearrange("(r p) k -> p k r", p=128))
            s = pm.tile([128, K], F32, tag="ars" + tag, name="ars" + tag)
            nc.vector.tensor_reduce(s[:], gsb[:].rearrange("p (k r) -> p k r", r=NC),
                                    axis=AX.X, op=ALU.add)
            return s

        # persistent weights
        w1s = pm.tile([128, 128], F32R)
        nc.gpsimd.memset(w1s[:].bitcast(F32), 0.0)
        nc.sync.dma_start(w1s[0:36, :], w1b)
        w2s = pm.tile([96, 192], F32R2);  nc.scalar.dma_start(w2s[:], w2)
        w3s = pm.tile([128, 768], BF16); nc.scalar.dma_start(w3s[:], w3p)
        wfcs = pm.tile([128, 6400], BF16, tag="wfcs", name="wfcs")
        nc.scalar.dma_start(wfcs[:], wfc)
        wfcos = pm.tile([128, 24], F32); nc.scalar.dma_start(wfcos[:], wfco)
        fcobs = pm.tile([12, 1], F32);   nc.scalar.dma_start(fcobs[:], fcob)
        ff4s = pm.tile([128, 32], F32);  nc.scalar.dma_start(ff4s[:], ff4)
        fb4s = pm.tile([32, 128], F32);  nc.scalar.dma_start(fb4s[:], fb4)
        ff2s = pm.tile([128, 64], F32);  nc.scalar.dma_start(ff2s[:], ff2)
        fb2s = pm.tile([64, 128], F32);  nc.scalar.dma_start(fb2s[:], fb2)

        # cross-stage tiles
        p2b = pm.tile([128, P2W], F32, tag="p2b", name="p2b")
        sump = pm.tile([128, 160], F32, tag="sump", name="sump")
        sqp = pm.tile([128, 160], F32, tag="sqp", name="sqp")
        ones = pm.tile([128, 320], F32, tag="ones", name="ones")
        nc.vector.memset(ones[:], 1.0)
        m2p = pm.tile([128, 16], F32, tag="m2p", name="m2p")
        m3p = pm.tile([128, 32], F32, tag="m3p", name="m3p")

        # ============ stage 1: conv1 -> stats + maxpool (42-padded out) ============
        with tc.tile_pool(name="pA", bufs=1) as pA:
          pooled = [pA.tile([128, 1764], F32, tag=f"pr{g}", name=f"pr{g}")
                    for g in range(8)]
          for g in range(8):
              nc.gpsimd.memset(pooled[g][:], 0.0)
          with tc.tile_pool(name="p1", bufs=2) as p1, \
               tc.tile_pool(name="pp1", bufs=4, space="PSUM") as pp1:
            for g in range(8):
                im = p1.tile([128, 6400], F32R, tag="im", name="im")
                nc.sync.dma_start(im[:], xim[g])
                for k in range(20):
                    ps = pp1.tile([128, 320], F32, tag="ps", name="c1ps")
                    nc.tensor.matmul(ps[:], w1s[:], im[:, 320 * k:320 * k + 320],
                                     start=True, stop=True)
                    idx = g * 20 + k
                    # evacuate PSUM once on Vector, folding in the sum for BN1
                    hb = p1.tile([128, 320], F32, tag="hb", name="hb")
                    nc.vector.scalar_tensor_tensor(hb[:], ps[:], 1.0, ones[:],
                                                   op0=ALU.mult, op1=ALU.mult,
                                                   accum_out=sump[:, idx:idx + 1])
                    # 2x2 maxpool: two pairwise-max steps from SBUF
                    px = p1.tile([128, 160], F32, tag="px", name="px")
                    w_ = hb[:].rearrange("p (y x two) -> p y x two", y=4, two=2)
                    nc.vector.tensor_tensor(px[:].rearrange("p (y x) -> p y x", y=4),
                                            w_[:, :, :, 0], w_[:, :, :, 1], op=ALU.max)
                    v_ = px[:].rearrange("p (y2 y x) -> p y2 y x", y2=2, y=2)
                    nc.vector.tensor_tensor(
                        pooled[g][:, 43 + 84 * k:43 + 84 * k + 84]
                            .rearrange("p (y x) -> p y x", x=42)[:, 0:2, 0:40],
                        v_[:, :, 0, :], v_[:, :, 1, :], op=ALU.max)
                    # square-sum on Scalar from SBUF (in-place square into hb)
                    nc.scalar.activation(hb[:], hb[:], ACTF.Square,
                                         accum_out=sqp[:, idx:idx + 1])

          # ---- BN1 stats: reduce partials, all-reduce, fold bands, broadcast ----
          st1 = pm.tile([128, 2], F32, tag="st1", name="st1")
          nc.vector.tensor_reduce(st1[:, 0:1], sump[:], axis=AX.X, op=ALU.add)
          nc.vector.tensor_reduce(st1[:, 1:2], sqp[:], axis=AX.X, op=ALU.add)
          sg1 = allred("1", st1[:], 2)
          f1s = pm.tile([32, 2], F32, tag="f1s", name="f1s")
          with tc.tile_pool(name="ppx1", bufs=1, space="PSUM") as ppx:
            psf = ppx.tile([32, 2], F32, tag="psf", name="psf")
            nc.tensor.matmul(psf[:], ff4s[:], sg1[:], start=True, stop=True)
            nc.scalar.copy(f1s[:], psf[:])
          m1 = pm.tile([32, 1], F32, tag="m1", name="m1")
          e2 = pm.tile([32, 1], F32, tag="e2", name="e2")
          nc.scalar.mul(m1[:], f1s[:, 0:1], 1.0 / N1)
          nc.scalar.mul(e2[:], f1s[:, 1:2], 1.0 / N1)
          v1 = pm.tile([32, 1], F32, tag="v1", name="v1")
          nc.vector.tensor_tensor(v1[:], m1[:], m1[:], op=ALU.mult)
          nc.vector.tensor_tensor(v1[:], e2[:], v1[:], op=ALU.subtract)
          nc.vector.tensor_scalar_add(v1[:], v1[:], EPS)
          sd = pm.tile([32, 1], F32, tag="sd", name="sd")
          nc.scalar.activation(sd[:], v1[:], ACTF.Sqrt, bias=0.0)
          rsb = pm.tile([32, 3], F32, tag="rsb", name="rsb")
          nc.vector.reciprocal(rsb[:, 0:1], sd[:])
          nc.vector.tensor_tensor(rsb[:, 1:2], m1[:], rsb[:, 0:1], op=ALU.mult)
          nc.scalar.mul(rsb[:, 1:2], rsb[:, 1:2], -1.0)
          nc.scalar.mul(rsb[:, 2:3], m1[:], -1.0)
          sc1b = pm.tile([128, 3], F32, tag="sc1b", name="sc1b")
          with tc.tile_pool(name="ppx2", bufs=1, space="PSUM") as ppx:
            psb = ppx.tile([128, 3], F32, tag="psb", name="psb")
            nc.tensor.matmul(psb[:], fb4s[:], rsb[:], start=True, stop=True)
            nc.scalar.copy(sc1b[:], psb[:])
          # fold the BN1 scale into the conv2 weights (rows are (dx, ch))
          nc.vector.tensor_scalar(w2s[:].bitcast(F32), w2s[:].bitcast(F32),
                                  sc1b[0:96, 0:1], None, op0=ALU.mult)
          if dbg:
              nc.sync.dma_start(dout("d_sc1b", [128, 2]), sc1b[:])
              nc.sync.dma_start(dout("d_st1", [128, 2]), st1[:])

          # ---- BN1 apply on vector: relu(x - mu); scale lives in w2s ----
          for g in range(8):
            vw = pooled[g][:, 43:43 + 1680].rearrange("p (y x) -> p y x", x=42)[:, :, 0:40]
            nc.vector.tensor_scalar(vw, vw, sc1b[:, 2:3], 0.0,
                                    op0=ALU.add, op1=ALU.max)
          if dbg:
              nc.sync.dma_start(dout("d_pool0", [128, 1764]), pooled[0][:])

          # ============ stage 2: conv2 -> mean + maxpool (22-padded out) ============
          with tc.tile_pool(name="p2", bufs=2) as p2, \
               tc.tile_pool(name="pp2", bufs=2, space="PSUM") as pp2:
            for g in range(8):
              a1c = p2.tile([96, A1W], F32R2, tag="a1c", name="a1c")
              if g < 2:
                  nc.gpsimd.memset(a1c[:].bitcast(F32), 0.0)
              for dx in range(3):
                for l in range(4):
                  ENG3[(dx * 4 + l) % 3].dma_start(
                      a1c[32 * dx:32 * dx + 32,
                          l * 1764 + 43 - dx:l * 1764 + 43 - dx + 1764],
                      pooled[g][32 * l:32 * l + 32, :].bitcast(F32R2))
              if dbg and g == 0:
                  nc.sync.dma_start(dout("d_a1c", [96, A1W]), a1c[:])
              for p in range(2):
                gp = g * 2 + p
                pc = pp2.tile([128, 2048], F32, tag="pcB", name="c2ps")
                for ck in range(4):
                    co, w = ck * 512, (512 if ck < 3 else 228)
                    for dyi in range(3):
                        for t2 in range(2):
                            base = 42 + (2 * p + t2) * 1764 + co + 42 * (dyi - 1)
                            nc.tensor.matmul(pc[64 * t2:64 * t2 + 64, co:co + w],
                                w2s[:, dyi * 64:dyi * 64 + 64],
                                a1c[:, base:base + w],
                                start=(dyi == 0), stop=(dyi == 2),
                                tile_position=(0, 64 * t2))
                vv = pc[:, 0:1764].rearrange("p (y x) -> p y x", x=42)[:, 1:41, 1:41]
                nc.vector.tensor_reduce(m2p[:, gp:gp + 1], vv, axis=AX.XY, op=ALU.add)
                nc.vector.tensor_reduce(
                    p2b[:, gp * 484 + 23:gp * 484 + 23 + 440]
                        .rearrange("p (y x) -> p y x", x=22)[:, :, 0:20],
                    vv.rearrange("p (y2 y) (x2 x) -> p y2 x2 y x", y=2, x=2),
                    axis=AX.XY, op=ALU.max)

        # ---- BN2 mean: all-reduce, fold 2 bands, broadcast ----
        l2c = pm.tile([128, 1], F32, tag="l2c", name="l2c")
        nc.vector.tensor_reduce(l2c[:], m2p[:], axis=AX.X, op=ALU.add)
        sg2 = allred("2", l2c[:], 1)
        bi2b = pm.tile([128, 1], F32, tag="bi2b", name="bi2b")
        with tc.tile_pool(name="ppx3", bufs=1, space="PSUM") as ppx:
            psf = ppx.tile([64, 1], F32, tag="ps2f", name="ps2f")
            nc.tensor.matmul(psf[:], ff2s[:], sg2[:], start=True, stop=True)
            f2s = pm.tile([64, 1], F32, tag="f2s", name="f2s")
            nc.scalar.mul(f2s[:], psf[:], -1.0 / 409600.0)
            psb = ppx.tile([128, 1], F32, tag="ps2b", name="ps2b")
            nc.tensor.matmul(psb[:], fb2s[:], f2s[:], start=True, stop=True)
            nc.scalar.copy(bi2b[:], psb[:])
        if dbg:
            nc.sync.dma_start(dout("d_bi2b", [128, 1]), bi2b[:])
            nc.sync.dma_start(dout("d_p2b", [128, P2W]), p2b[:])

        # ============ stage 3: sign2 -> conv3 -> mean + sign-pool ============
        p3b = pm.tile([128, 3200], F32, tag="p3b", name="p3b")
        with tc.tile_pool(name="p3", bufs=2) as p3pool, \
             tc.tile_pool(name="pp3", bufs=4, space="PSUM") as pp3:
          b2p = p3pool.tile([128, B2W], BF16, tag="b2p", name="b2p", bufs=1)
          nc.gpsimd.memset(b2p[:], 0.0)
          for i in range(B):
            gp, t2 = i // 2, i % 2
            src = p2b[64 * t2:64 * t2 + 64, gp * 484 + 23:gp * 484 + 23 + 440] \
                .rearrange("p (y x) -> p y x", x=22)[:, :, 0:20]
            dst = b2p[0:64, 24 + i * 484 + 23: 24 + i * 484 + 23 + 440] \
                .rearrange("p (y x) -> p y x", x=22)[:, :, 0:20]
            nc.scalar.activation(dst, src, ACTF.Sign,
                                 bias=bi2b[64 * t2:64 * t2 + 64, :])
          HW2 = (B2W - 22) // 2
          nc.sync.dma_start(b2p[64:128, 0:HW2], b2p[0:64, 22:22 + HW2])
          nc.scalar.dma_start(b2p[64:128, HW2:B2W - 22], b2p[0:64, 22 + HW2:B2W])
          for i in range(B):
            p3 = pp3.tile([128, 484], F32, tag="p3ps", name="c3ps")
            for dx in range(3):
                bp = 24 + i * 484 - 22 + (dx - 1)
                nc.tensor.matmul(p3[:], w3s[:, dx * 128:(dx + 1) * 128],
                                 b2p[:, bp:bp + 484],
                                 start=(dx == 0), stop=False)
            for dx in range(3):
                bs = 24 + i * 484 + 22 + (dx - 1)
                nc.tensor.matmul(p3[:], w3s[0:64, 384 + dx * 128:384 + (dx + 1) * 128],
                                 b2p[0:64, bs:bs + 484],
                                 start=False, stop=(dx == 2))
            vv = p3[:].rearrange("p (y x) -> p y x", x=22)[:, 1:21, 1:21]
            nc.vector.tensor_reduce(m3p[:, i:i + 1], vv, axis=AX.XY, op=ALU.add)
            nc.vector.tensor_reduce(
                p3b[:, i * 100:(i + 1) * 100].rearrange("p (y x) -> p y x", y=10),
                vv.rearrange("p (y2 y) (x2 x) -> p y2 x2 y x", y=2, x=2),
                axis=AX.XY, op=ALU.max)

        l3c = pm.tile([128, 1], F32, tag="l3c", name="l3c")
        nc.vector.tensor_reduce(l3c[:], m3p[:], axis=AX.X, op=ALU.add)
        sg3 = allred("3", l3c[:], 1)
        nc.scalar.mul(sg3[:], sg3[:], -1.0 / 102400.0)
        if dbg:
            nc.sync.dma_start(dout("d_sg3", [128, 1]), sg3[:])
            nc.sync.dma_start(dout("d_p3b", [128, 3200]), p3b[:])

        # ============ stage 4: sign3, avgpool, fc1, bn1d sign, fco ============
        with tc.tile_pool(name="p4", bufs=1) as p4, \
             tc.tile_pool(name="pp4", bufs=1, space="PSUM") as pp4:
          s3t = p4.tile([128, 3200], BF16, tag="s3t", name="s3t")
          nc.scalar.activation(s3t[:], p3b[:], ACTF.Sign, bias=sg3[:])
          zx = p4.tile([128, 1600], BF16, tag="zx", name="zx")
          v = s3t[:].rearrange("p (a x) -> p a x", x=2)
          nc.vector.tensor_tensor(zx[:], v[:, :, 0], v[:, :, 1], op=ALU.add)
          z2 = p4.tile([128, 800], BF16, tag="z2", name="z2")
          u = zx[:].rearrange("p (i y2 y x) -> p i y2 y x", i=32, y2=5, y=2)
          nc.vector.tensor_tensor(z2[:].rearrange("p (i y x) -> p i y x", i=32, y=5),
                                  u[:, :, :, 0, :], u[:, :, :, 1, :], op=ALU.add)
          zr = z2[:].rearrange("p (i s) -> p s i", s=25)
          mfp = pm.tile([128, 2], F32, tag="mfp", name="mfp")
          zss = []
          for hh in range(2):
              zs = pp4.tile([128, 32], F32, tag=f"zs{hh}", name=f"fcps{hh}")
              for sp in range(25):
                  nc.tensor.matmul(zs[:], wfcs[:, sp * 256 + 128 * hh: sp * 256 + 128 * hh + 128],
                                   zr[:, sp, :], start=(sp == 0), stop=(sp == 24))
              nc.vector.tensor_reduce(mfp[:, hh:hh + 1], zs[:], axis=AX.X, op=ALU.add)
              zss.append(zs)
          g4 = allred("4", mfp[:], 2)
          nc.scalar.mul(g4[:], g4[:], -1.0 / 256.0)
          ssb = pm.tile([128, 64], F32, tag="ssb", name="ssb")
          for hh in range(2):
              nc.scalar.activation(ssb[:, 32 * hh:32 * hh + 32], zss[hh][:],
                                   ACTF.Sign, bias=g4[:, hh:hh + 1])
          po = pp4.tile([12, 32], F32, tag="po", name="fops")
          for hh in range(2):
              nc.tensor.matmul(po[:], wfcos[:, hh * 12:hh * 12 + 12],
                               ssb[:, 32 * hh:32 * hh + 32],
                               start=(hh == 0), stop=(hh == 1))
          osb = pm.tile([12, 32], F32, tag="osb", name="osb")
          nc.scalar.activation(osb[:], po[:], ACTF.Identity, bias=fcobs[:], scale=1.0)
          nc.sync.dma_start(out, osb[:])

    nc.compile()
    return nc


def _prep_host(inputs):
    x = np.asarray(inputs["x"], np.float32)
    w1 = np.asarray(inputs["conv1_w"], np.float32)
    w2 = np.sign(np.asarray(inputs["w2"], np.float32))
    w3 = np.sign(np.asarray(inputs["w3"], np.float32))
    fc1 = np.sign(np.asarray(inputs["fc1_w"], np.float32))
    fco_w = np.asarray(inputs["fco_w"], np.float32)
    fco_b = np.asarray(inputs["fco_b"], np.float32)

    w1b = np.zeros((36, 128), np.float32)
    for r in range(4):
        w1b[9 * r:9 * r + 9, 32 * r:32 * r + 32] = w1[:, 0].reshape(32, 9).T
    w2m = np.zeros((96, 192), np.float32)
    for dyi in range(3):
        for dxi in range(3):
            w2m[32 * dxi:32 * dxi + 32, dyi * 64:(dyi + 1) * 64] = w2[:, :, dyi, dxi].T
    w3m = np.zeros((128, 768), ml_dtypes.bfloat16)
    for dx in range(3):
        w3m[0:64, dx * 128:(dx + 1) * 128] = w3[:, :, 0, dx].T.astype(ml_dtypes.bfloat16)
        w3m[64:128, dx * 128:(dx + 1) * 128] = w3[:, :, 1, dx].T.astype(ml_dtypes.bfloat16)
        w3m[0:64, 384 + dx * 128:384 + (dx + 1) * 128] = w3[:, :, 2, dx].T.astype(ml_dtypes.bfloat16)
    wfcm = np.zeros((128, 6400), ml_dtypes.bfloat16)
    fc1r = fc1.reshape(256, 128, 25)
    for sp in range(25):
        wfcm[:, sp * 256:(sp + 1) * 256] = fc1r[:, :, sp].T.astype(ml_dtypes.bfloat16)
    wfcom = np.zeros((128, 24), np.float32)
    wfcom[:, 0:12] = fco_w[:, 0:128].T
    wfcom[:, 12:24] = fco_w[:, 128:256].T
    fcobm = fco_b.reshape(12, 1).astype(np.float32)

    ff4 = np.zeros((128, 32), np.float32)
    for r in range(4):
        ff4[32 * r:32 * r + 32, :] = np.eye(32, dtype=np.float32)
    fb4 = ff4.T.copy()
    ff2 = np.zeros((128, 64), np.float32)
    for r in range(2):
        ff2[64 * r:64 * r + 64, :] = np.eye(64, dtype=np.float32)
    fb2 = ff2.T.copy()

    in_maps = []
    for core in range(NC):
        xs = x[core * B:(core + 1) * B, 0]
        xpad = np.zeros((B, 82, 82), np.float32)
        xpad[:, 1:81, 1:81] = xs
        win = np.lib.stride_tricks.sliding_window_view(xpad, (80, 80), axis=(1, 2))
        xim = np.zeros((8, 128, 6400), np.float32)
        xim[:, 0:36, :] = win.reshape(8, 36, 6400)
        in_maps.append({"xim": xim, "w1b": w1b, "w2": w2m, "w3p": w3m,
                        "wfc": wfcm, "wfco": wfcom, "fcob": fcobm,
                        "ff4": ff4, "fb4": fb4, "ff2": ff2, "fb2": fb2})
    return in_maps


def kernel(**inputs):
    dbg = bool(int(os.environ.get("BNN_DEBUG", "0")))
    if "nc" not in _cache:
        _cache["nc"] = _build(dbg=dbg)
    nc = _cache["nc"]
    in_maps = _prep_host(inputs)
    trace = bool(int(os.environ.get("BNN_TRACE", "0")))
    if trace:
        sys.path.insert(0, os.path.dirname(os.path.abspath(__file__)))
        try:
            import ntff_shim
            ntff_shim.install()
        except Exception:
            pass
    tdir = os.environ.get("BNN_TRACE_DIR") if trace else None
    res = bass_utils.run_bass_kernel_spmd(nc, in_maps, core_ids=list(range(NC)), trace=trace,
                                          tmpdir=tdir)
    _cache["exec_time_ns"] = res.exec_time_ns
    if trace and res.instructions_and_trace:
        _cache["trace_path"] = res.instructions_and_trace[1]
    _cache["results"] = res.results
    out = np.zeros((256, 12), np.float32)
    for core in range(NC):
        out[core * B:(core + 1) * B, :] = res.results[core]["out"].T
    return out


# revision 34
# speedup vs baseline: 1.0005x; 1.0005x over previous
import sys, os
sys.path.insert(0, '/opt/trn_rl_repo')
import numpy as np
import ml_dtypes

import concourse.bass as bass
import concourse.bacc as bacc
import concourse.tile as tile
from concourse import mybir, bass_utils

F32 = mybir.dt.float32
_f32r_lvl = int(os.environ.get("BNN_F32R", "0"))
F32R = mybir.dt.float32r if _f32r_lvl >= 1 else mybir.dt.float32
F32R2 = mybir.dt.float32r if _f32r_lvl >= 2 else mybir.dt.float32
BF16 = mybir.dt.bfloat16
ACTF = mybir.ActivationFunctionType
ALU = mybir.AluOpType
AX = mybir.AxisListType
NC, B, EPS = 8, 32, 1e-5
RG = [list(range(NC))]
_cache = {}

A1W = 7144                      # 4 images of padded 42x42 + slack
P2W = 16 * 484                  # stage-2 pooled, 22-padded per image
B2W = 24 + B * 484 + 24         # stage-3 input row width
N1 = 256.0 * 6400.0


def _build(dbg=False):
    nc = bacc.Bacc("TRN2", target_bir_lowering=False, debug=False, num_devices=NC)

    def din(name, shape, dt=F32):
        return nc.dram_tensor(name, list(shape), dt, kind="ExternalInput").ap()

    dbg_outs = {}
    def dout(name, shape, dt=F32):
        dbg_outs[name] = nc.dram_tensor(name, list(shape), dt, kind="ExternalOutput").ap()
        return dbg_outs[name]

    xim  = din("xim", [8, 128, 6400], F32R)
    w1b  = din("w1b", [36, 128], F32R)
    w2   = din("w2", [96, 192], F32R2)
    w3p  = din("w3p", [128, 768], BF16)
    wfc  = din("wfc", [128, 6400], BF16)
    wfco = din("wfco", [128, 24])
    fcob = din("fcob", [12, 1])
    ff4  = din("ff4", [128, 32])
    fb4  = din("fb4", [32, 128])
    ff2  = din("ff2", [128, 64])
    fb2  = din("fb2", [64, 128])
    out  = nc.dram_tensor("out", [12, B], F32, kind="ExternalOutput").ap()

    ENG3 = None  # set inside context

    with tile.TileContext(nc) as tc:
      with tc.tile_pool(name="pm", bufs=1) as pm, \
           tc.tile_pool(name="pd", bufs=1, space="DRAM") as pd:
        ENG3 = [nc.sync, nc.scalar, nc.gpsimd]

        def allred(tag, src, K):
            bi = pd.tile([128, K], F32, tag="ari" + tag, name="ari" + tag)
            bo = pd.tile([128 * NC, K], F32, tag="aro" + tag, name="aro" + tag)
            nc.sync.dma_start(bi[:], src)
            nc.gpsimd.collective_compute("AllGather", ALU.bypass, replica_groups=RG,
                                         ins=[bi.opt()], outs=[bo.opt()])
            gsb = pm.tile([128, K * NC], F32, tag="arg" + tag, name="arg" + tag)
            nc.sync.dma_start(gsb[:].rearrange("p (k r) -> p k r", r=NC),
                              bo[:].rearrange("(r p) k -> p k r", p=128))
            s = pm.tile([128, K], F32, tag="ars" + tag, name="ars" + tag)
            nc.vector.tensor_reduce(s[:], gsb[:].rearrange("p (k r) -> p k r", r=NC),
                                    axis=AX.X, op=ALU.add)
            return s

        # persistent weights
        w1s = pm.tile([128, 128], F32R)
        nc.gpsimd.memset(w1s[:].bitcast(F32), 0.0)
        nc.sync.dma_start(w1s[0:36, :], w1b)
        w2s = pm.tile([96, 192], F32R2);  nc.scalar.dma_start(w2s[:], w2)
        w3s = pm.tile([128, 768], BF16); nc.scalar.dma_start(w3s[:], w3p)
        wfcs = pm.tile([128, 6400], BF16, tag="wfcs", name="wfcs")
        nc.scalar.dma_start(wfcs[:], wfc)
        wfcos = pm.tile([128, 24], F32); nc.scalar.dma_start(wfcos[:], wfco)
        fcobs = pm.tile([12, 1], F32);   nc.scalar.dma_start(fcobs[:], fcob)
        ff4s = pm.tile([128, 32], F32);  nc.scalar.dma_start(ff4s[:], ff4)
        fb4s = pm.tile([32, 128], F32);  nc.scalar.dma_start(fb4s[:], fb4)
        ff2s = pm.tile([128, 64], F32);  nc.scalar.dma_start(ff2s[:], ff2)
        fb2s = pm.tile([64, 128], F32);  nc.scalar.dma_start(fb2s[:], fb2)

        # cross-stage tiles
        p2b = pm.tile([128, P2W], F32, tag="p2b", name="p2b")
        sump = pm.tile([128, 160], F32, tag="sump", name="sump")
        sqp = pm.tile([128, 160], F32, tag="sqp", name="sqp")
        ones = pm.tile([128, 320], F32, tag="ones", name="ones")
        nc.vector.memset(ones[:], 1.0)
        m2p = pm.tile([128, 16], F32, tag="m2p", name="m2p")
        m3p = pm.tile([128, 32], F32, tag="m3p", name="m3p")

        # ============ stage 1: conv1 -> stats + maxpool (42-padded out) ============
        with tc.tile_pool(name="pA", bufs=1) as pA:
          pooled = [pA.tile([128, 1764], F32, tag=f"pr{g}", name=f"pr{g}")
                    for g in range(8)]
          for g in range(8):
              nc.gpsimd.memset(pooled[g][:], 0.0)
          with tc.tile_pool(name="p1", bufs=2) as p1, \
               tc.tile_pool(name="pp1", bufs=4, space="PSUM") as pp1:
            for g in range(8):
                im = p1.tile([128, 6400], F32R, tag="im", name="im")
                nc.sync.dma_start(im[:], xim[g])
                for k in range(20):
                    ps = pp1.tile([128, 320], F32, tag="ps", name="c1ps")
                    nc.tensor.matmul(ps[:], w1s[:], im[:, 320 * k:320 * k + 320],
                                     start=True, stop=True)
                    idx = g * 20 + k
                    # evacuate PSUM once on Vector, folding in the sum for BN1
                    hb = p1.tile([128, 320], F32, tag="hb", name="hb")
                    nc.vector.scalar_tensor_tensor(hb[:], ps[:], 1.0, ones[:],
                                                   op0=ALU.mult, op1=ALU.mult,
                                                   accum_out=sump[:, idx:idx + 1])
                    # 2x2 maxpool: two pairwise-max steps from SBUF
                    px = p1.tile([128, 160], F32, tag="px", name="px")
                    w_ = hb[:].rearrange("p (y x two) -> p y x two", y=4, two=2)
                    nc.vector.tensor_tensor(px[:].rearrange("p (y x) -> p y x", y=4),
                                            w_[:, :, :, 0], w_[:, :, :, 1], op=ALU.max)
                    v_ = px[:].rearrange("p (y2 y x) -> p y2 y x", y2=2, y=2)
                    nc.vector.tensor_tensor(
                        pooled[g][:, 43 + 84 * k:43 + 84 * k + 84]
                            .rearrange("p (y x) -> p y x", x=42)[:, 0:2, 0:40],
                        v_[:, :, 0, :], v_[:, :, 1, :], op=ALU.max)
                    # square-sum on Scalar from SBUF (in-place square into hb)
                    nc.scalar.activation(hb[:], hb[:], ACTF.Square,
                                         accum_out=sqp[:, idx:idx + 1])

          # ---- BN1 stats: reduce partials, all-reduce, fold bands, broadcast ----
          st1 = pm.tile([128, 2], F32, tag="st1", name="st1")
          nc.vector.tensor_reduce(st1[:, 0:1], sump[:], axis=AX.X, op=ALU.add)
          nc.vector.tensor_reduce(st1[:, 1:2], sqp[:], axis=AX.X, op=ALU.add)
          sg1 = allred("1", st1[:], 2)
          f1s = pm.tile([32, 2], F32, tag="f1s", name="f1s")
          with tc.tile_pool(name="ppx1", bufs=1, space="PSUM") as ppx:
            psf = ppx.tile([32, 2], F32, tag="psf", name="psf")
            nc.tensor.matmul(psf[:], ff4s[:], sg1[:], start=True, stop=True)
            nc.scalar.copy(f1s[:], psf[:])
          m1 = pm.tile([32, 1], F32, tag="m1", name="m1")
          e2 = pm.tile([32, 1], F32, tag="e2", name="e2")
          nc.scalar.mul(m1[:], f1s[:, 0:1], 1.0 / N1)
          nc.scalar.mul(e2[:], f1s[:, 1:2], 1.0 / N1)
          v1 = pm.tile([32, 1], F32, tag="v1", name="v1")
          nc.vector.tensor_tensor(v1[:], m1[:], m1[:], op=ALU.mult)
          nc.vector.tensor_tensor(v1[:], e2[:], v1[:], op=ALU.subtract)
          nc.vector.tensor_scalar_add(v1[:], v1[:], EPS)
          sd = pm.tile([32, 1], F32, tag="sd", name="sd")
          nc.scalar.activation(sd[:], v1[:], ACTF.Sqrt, bias=0.0)
          rsb = pm.tile([32, 3], F32, tag="rsb", name="rsb")
          nc.vector.reciprocal(rsb[:, 0:1], sd[:])
          nc.vector.tensor_tensor(rsb[:, 1:2], m1[:], rsb[:, 0:1], op=ALU.mult)
          nc.scalar.mul(rsb[:, 1:2], rsb[:, 1:2], -1.0)
          nc.scalar.mul(rsb[:, 2:3], m1[:], -1.0)
          sc1b = pm.tile([128, 3], F32, tag="sc1b", name="sc1b")
          with tc.tile_pool(name="ppx2", bufs=1, space="PSUM") as ppx:
            psb = ppx.tile([128, 3], F32, tag="psb", name="psb")
            nc.tensor.matmul(psb[:], fb4s[:], rsb[:], start=True, stop=True)
            nc.scalar.copy(sc1b[:], psb[:])
          # fold the BN1 scale into the conv2 weights (rows are (dx, ch))
          nc.vector.tensor_scalar(w2s[:].bitcast(F32), w2s[:].bitcast(F32),
                                  sc1b[0:96, 0:1], None, op0=ALU.mult)
          if dbg:
              nc.sync.dma_start(dout("d_sc1b", [128, 2]), sc1b[:])
              nc.sync.dma_start(dout("d_st1", [128, 2]), st1[:])

          # ---- BN1 apply on vector: relu(x - mu); scale lives in w2s ----
          for g in range(8):
            vw = pooled[g][:, 43:43 + 1680].rearrange("p (y x) -> p y x", x=42)[:, :, 0:40]
            nc.vector.tensor_scalar(vw, vw, sc1b[:, 2:3], 0.0,
                                    op0=ALU.add, op1=ALU.max)
          if dbg:
              nc.sync.dma_start(dout("d_pool0", [128, 1764]), pooled[0][:])

          # ============ stage 2: conv2 -> mean + maxpool (22-padded out) ============
          with tc.tile_pool(name="p2", bufs=2) as p2, \
               tc.tile_pool(name="pp2", bufs=2, space="PSUM") as pp2:
            for g in range(8):
              a1c = p2.tile([96, A1W], F32R2, tag="a1c", name="a1c")
              if g < 2:
                  nc.gpsimd.memset(a1c[:].bitcast(F32), 0.0)
              for dx in range(3):
                for l in range(4):
                  ENG3[(dx * 4 + l) % 3].dma_start(
                      a1c[32 * dx:32 * dx + 32,
                          l * 1764 + 43 - dx:l * 1764 + 43 - dx + 1764],
                      pooled[g][32 * l:32 * l + 32, :].bitcast(F32R2))
              if dbg and g == 0:
                  nc.sync.dma_start(dout("d_a1c", [96, A1W]), a1c[:])
              for p in range(2):
                gp = g * 2 + p
                pc = pp2.tile([128, 2048], F32, tag="pcB", name="c2ps")
                for ck in range(4):
                    co, w = ck * 512, (512 if ck < 3 else 144)
                    for dyi in range(3):
                        for t2 in range(2):
                            base = 84 + (2 * p + t2) * 1764 + co + 42 * (dyi - 1)
                            nc.tensor.matmul(pc[64 * t2:64 * t2 + 64, co:co + w],
                                w2s[:, dyi * 64:dyi * 64 + 64],
                                a1c[:, base:base + w],
                                start=(dyi == 0), stop=(dyi == 2),
                                tile_position=(0, 64 * t2))
                vv = pc[:, 0:1680].rearrange("p (y x) -> p y x", x=42)[:, :, 1:41]
                nc.vector.tensor_reduce(m2p[:, gp:gp + 1], vv, axis=AX.XY, op=ALU.add)
                nc.vector.tensor_reduce(
                    p2b[:, gp * 484 + 23:gp * 484 + 23 + 440]
                        .rearrange("p (y x) -> p y x", x=22)[:, :, 0:20],
                    vv.rearrange("p (y2 y) (x2 x) -> p y2 x2 y x", y=2, x=2),
                    axis=AX.XY, op=ALU.max)

        # ---- BN2 mean: all-reduce, fold 2 bands, broadcast ----
        l2c = pm.tile([128, 1], F32, tag="l2c", name="l2c")
        nc.vector.tensor_reduce(l2c[:], m2p[:], axis=AX.X, op=ALU.add)
        sg2 = allred("2", l2c[:], 1)
        bi2b = pm.tile([128, 1], F32, tag="bi2b", name="bi2b")
        with tc.tile_pool(name="ppx3", bufs=1, space="PSUM") as ppx:
            psf = ppx.tile([64, 1], F32, tag="ps2f", name="ps2f")
            nc.tensor.matmul(psf[:], ff2s[:], sg2[:], start=True, stop=True)
            f2s = pm.tile([64, 1], F32, tag="f2s", name="f2s")
            nc.scalar.mul(f2s[:], psf[:], -1.0 / 409600.0)
            psb = ppx.tile([128, 1], F32, tag="ps2b", name="ps2b")
            nc.tensor.matmul(psb[:], fb2s[:], f2s[:], start=True, stop=True)
            nc.scalar.copy(bi2b[:], psb[:])
        if dbg:
            nc.sync.dma_start(dout("d_bi2b", [128, 1]), bi2b[:])
            nc.sync.dma_start(dout("d_p2b", [128, P2W]), p2b[:])

        # ============ stage 3: sign2 -> conv3 -> mean + sign-pool ============
        p3b = pm.tile([128, 3200], F32, tag="p3b", name="p3b")
        with tc.tile_pool(name="p3", bufs=2) as p3pool, \
             tc.tile_pool(name="pp3", bufs=4, space="PSUM") as pp3:
          b2p = p3pool.tile([128, B2W], BF16, tag="b2p", name="b2p", bufs=1)
          nc.gpsimd.memset(b2p[:], 0.0)
          for i in range(B):
            gp, t2 = i // 2, i % 2
            src = p2b[64 * t2:64 * t2 + 64, gp * 484 + 23:gp * 484 + 23 + 440] \
                .rearrange("p (y x) -> p y x", x=22)[:, :, 0:20]
            dst = b2p[0:64, 24 + i * 484 + 23: 24 + i * 484 + 23 + 440] \
                .rearrange("p (y x) -> p y x", x=22)[:, :, 0:20]
            nc.scalar.activation(dst, src, ACTF.Sign,
                                 bias=bi2b[64 * t2:64 * t2 + 64, :])
          for i in range(B):
            c0 = 24 + i * 484 - 2
            ENG3[i % 2].dma_start(b2p[64:128, c0:c0 + 444], b2p[0:64, c0 + 22:c0 + 466])
          for i in range(B):
            p3 = pp3.tile([128, 440], F32, tag="p3ps", name="c3ps")
            for dx in range(3):
                bp = 24 + i * 484 + (dx - 1)
                nc.tensor.matmul(p3[:], w3s[:, dx * 128:(dx + 1) * 128],
                                 b2p[:, bp:bp + 440],
                                 start=(dx == 0), stop=False)
            for dx in range(3):
                bs = 24 + i * 484 + 44 + (dx - 1)
                nc.tensor.matmul(p3[:], w3s[0:64, 384 + dx * 128:384 + (dx + 1) * 128],
                                 b2p[0:64, bs:bs + 440],
                                 start=False, stop=(dx == 2))
            vv = p3[:, 0:440].rearrange("p (y x) -> p y x", x=22)[:, :, 1:21]
            nc.vector.tensor_reduce(m3p[:, i:i + 1], vv, axis=AX.XY, op=ALU.add)
            nc.vector.tensor_reduce(
                p3b[:, i * 100:(i + 1) * 100].rearrange("p (y x) -> p y x", y=10),
                vv.rearrange("p (y2 y) (x2 x) -> p y2 x2 y x", y=2, x=2),
                axis=AX.XY, op=ALU.max)

        l3c = pm.tile([128, 1], F32, tag="l3c", name="l3c")
        nc.vector.tensor_reduce(l3c[:], m3p[:], axis=AX.X, op=ALU.add)
        sg3 = allred("3", l3c[:], 1)
        nc.scalar.mul(sg3[:], sg3[:], -1.0 / 102400.0)
        if dbg:
            nc.sync.dma_start(dout("d_sg3", [128, 1]), sg3[:])
            nc.sync.dma_start(dout("d_p3b", [128, 3200]), p3b[:])

        # ============ stage 4: sign3, avgpool, fc1, bn1d sign, fco ============
        with tc.tile_pool(name="p4", bufs=1) as p4, \
             tc.tile_pool(name="pp4", bufs=1, space="PSUM") as pp4:
          s3t = p4.tile([128, 3200], BF16, tag="s3t", name="s3t")
          nc.scalar.activation(s3t[:], p3b[:], ACTF.Sign, bias=sg3[:])
          zx = p4.tile([128, 1600], BF16, tag="zx", name="zx")
          v = s3t[:].rearrange("p (a x) -> p a x", x=2)
          nc.vector.tensor_tensor(zx[:], v[:, :, 0], v[:, :, 1], op=ALU.add)
          z2 = p4.tile([128, 800], BF16, tag="z2", name="z2")
          u = zx[:].rearrange("p (i y2 y x) -> p i y2 y x", i=32, y2=5, y=2)
          nc.vector.tensor_tensor(z2[:].rearrange("p (i y x) -> p i y x", i=32, y=5),
                                  u[:, :, :, 0, :], u[:, :, :, 1, :], op=ALU.add)
          zr = z2[:].rearrange("p (i s) -> p s i", s=25)
          mfp = pm.tile([128, 2], F32, tag="mfp", name="mfp")
          zss = []
          for hh in range(2):
              zs = pp4.tile([128, 32], F32, tag=f"zs{hh}", name=f"fcps{hh}")
              for sp in range(25):
                  nc.tensor.matmul(zs[:], wfcs[:, sp * 256 + 128 * hh: sp * 256 + 128 * hh + 128],
                                   zr[:, sp, :], start=(sp == 0), stop=(sp == 24))
              nc.vector.tensor_reduce(mfp[:, hh:hh + 1], zs[:], axis=AX.X, op=ALU.add)
              zss.append(zs)
          g4 = allred("4", mfp[:], 2)
          nc.scalar.mul(g4[:], g4[:], -1.0 / 256.0)
          ssb = pm.tile([128, 64], F32, tag="ssb", name="ssb")
          for hh in range(2):
              nc.scalar.activation(ssb[:, 32 * hh:32 * hh + 32], zss[hh][:],
                                   ACTF.Sign, bias=g4[:, hh:hh + 1])
          po = pp4.tile([12, 32], F32, tag="po", name="fops")
          for hh in range(2):
              nc.tensor.matmul(po[:], wfcos[:, hh * 12:hh * 12 + 12],
                               ssb[:, 32 * hh:32 * hh + 32],
                               start=(hh == 0), stop=(hh == 1))
          osb = pm.tile([12, 32], F32, tag="osb", name="osb")
          nc.scalar.activation(osb[:], po[:], ACTF.Identity, bias=fcobs[:], scale=1.0)
          nc.sync.dma_start(out, osb[:])

    nc.compile()
    return nc


def _prep_host(inputs):
    x = np.asarray(inputs["x"], np.float32)
    w1 = np.asarray(inputs["conv1_w"], np.float32)
    w2 = np.sign(np.asarray(inputs["w2"], np.float32))
    w3 = np.sign(np.asarray(inputs["w3"], np.float32))
    fc1 = np.sign(np.asarray(inputs["fc1_w"], np.float32))
    fco_w = np.asarray(inputs["fco_w"], np.float32)
    fco_b = np.asarray(inputs["fco_b"], np.float32)

    w1b = np.zeros((36, 128), np.float32)
    for r in range(4):
        w1b[9 * r:9 * r + 9, 32 * r:32 * r + 32] = w1[:, 0].reshape(32, 9).T
    w2m = np.zeros((96, 192), np.float32)
    for dyi in range(3):
        for dxi in range(3):
            w2m[32 * dxi:32 * dxi + 32, dyi * 64:(dyi + 1) * 64] = w2[:, :, dyi, dxi].T
    w3m = np.zeros((128, 768), ml_dtypes.bfloat16)
    for dx in range(3):
        w3m[0:64, dx * 128:(dx + 1) * 128] = w3[:, :, 0, dx].T.astype(ml_dtypes.bfloat16)
        w3m[64:128, dx * 128:(dx + 1) * 128] = w3[:, :, 1, dx].T.astype(ml_dtypes.bfloat16)
        w3m[0:64, 384 + dx * 128:384 + (dx + 1) * 128] = w3[:, :, 2, dx].T.astype(ml_dtypes.bfloat16)
    wfcm = np.zeros((128, 6400), ml_dtypes.bfloat16)
    fc1r = fc1.reshape(256, 128, 25)
    for sp in range(25):
        wfcm[:, sp * 256:(sp + 1) * 256] = fc1r[:, :, sp].T.astype(ml_dtypes.bfloat16)
    wfcom = np.zeros((128, 24), np.float32)
    wfcom[:, 0:12] = fco_w[:, 0:128].T
    wfcom[:, 12:24] = fco_w[:, 128:256].T
    fcobm = fco_b.reshape(12, 1).astype(np.float32)

    ff4 = np.zeros((128, 32), np.float32)
    for r in range(4):
        ff4[32 * r:32 * r + 32, :] = np.eye(32, dtype=np.float32)
    fb4 = ff4.T.copy()
    ff2 = np.zeros((128, 64), np.float32)
    for r in range(2):
        ff2[64 * r:64 * r + 64, :] = np.eye(64, dtype=np.float32)
    fb2 = ff2.T.copy()

    in_maps = []
    for core in range(NC):
        xs = x[core * B:(core + 1) * B, 0]
        xpad = np.zeros((B, 82, 82), np.float32)
        xpad[:, 1:81, 1:81] = xs
        win = np.lib.stride_tricks.sliding_window_view(xpad, (80, 80), axis=(1, 2))
        xim = np.zeros((8, 128, 6400), np.float32)
        xim[:, 0:36, :] = win.reshape(8, 36, 6400)
        in_maps.append({"xim": xim, "w1b": w1b, "w2": w2m, "w3p": w3m,
                        "wfc": wfcm, "wfco": wfcom, "fcob": fcobm,
                        "ff4": ff4, "fb4": fb4, "ff2": ff2, "fb2": fb2})
    return in_maps


def kernel(**inputs):
    dbg = bool(int(os.environ.get("BNN_DEBUG", "0")))
    if "nc" not in _cache:
        _cache["nc"] = _build(dbg=dbg)
    nc = _cache["nc"]
    in_maps = _prep_host(inputs)
    trace = bool(int(os.environ.get("BNN_TRACE", "0")))
    if trace:
        sys.path.insert(0, os.path.dirname(os.path.abspath(__file__)))
        try:
            import ntff_shim
            ntff_shim.install()
        except Exception:
            pass
    tdir = os.environ.get("BNN_TRACE_DIR") if trace else None
    res = bass_utils.run_bass_kernel_spmd(nc, in_maps, core_ids=list(range(NC)), trace=trace,
                                          tmpdir=tdir)
    _cache["exec_time_ns"] = res.exec_time_ns
    if trace and res.instructions_and_trace:
        _cache["trace_path"] = res.instructions_and_trace[1]
    _cache["results"] = res.results
    out = np.zeros((256, 12), np.float32)
    for core in range(NC):
        out[core * B:(core + 1) * B, :] = res.results[core]["out"].T
    return out


# revision 35
# speedup vs baseline: 1.0815x; 1.0810x over previous
import sys, os
sys.path.insert(0, '/opt/trn_rl_repo')
import numpy as np
import ml_dtypes

import concourse.bass as bass
import concourse.bacc as bacc
import concourse.tile as tile
from concourse import mybir, bass_utils

F32 = mybir.dt.float32
_f32r_lvl = int(os.environ.get("BNN_F32R", "0"))
F32R = mybir.dt.float32r if _f32r_lvl >= 1 else mybir.dt.float32
F32R2 = mybir.dt.float32r if _f32r_lvl >= 2 else mybir.dt.float32
BF16 = mybir.dt.bfloat16
ACTF = mybir.ActivationFunctionType
ALU = mybir.AluOpType
AX = mybir.AxisListType
NC, B, EPS = 8, 32, 1e-5
RG = [list(range(NC))]
_cache = {}

A1W = 7144                      # 4 images of padded 42x42 + slack
P2W = 16 * 484                  # stage-2 pooled, 22-padded per image
B2W = 24 + B * 484 + 24         # stage-3 input row width
N1 = 256.0 * 6400.0


def _build(dbg=False):
    nc = bacc.Bacc("TRN2", target_bir_lowering=False, debug=False, num_devices=NC)

    def din(name, shape, dt=F32):
        return nc.dram_tensor(name, list(shape), dt, kind="ExternalInput").ap()

    dbg_outs = {}
    def dout(name, shape, dt=F32):
        dbg_outs[name] = nc.dram_tensor(name, list(shape), dt, kind="ExternalOutput").ap()
        return dbg_outs[name]

    xim  = din("xim", [8, 128, 6400], F32R)
    w1b  = din("w1b", [36, 128], F32R)
    w2   = din("w2", [96, 192], F32R2)
    w3p  = din("w3p", [128, 768], BF16)
    wfc  = din("wfc", [128, 6400], BF16)
    wfco = din("wfco", [128, 24])
    fcob = din("fcob", [12, 1])
    ff4  = din("ff4", [128, 32])
    fb4  = din("fb4", [32, 128])
    ff2  = din("ff2", [128, 64])
    fb2  = din("fb2", [64, 128])
    out  = nc.dram_tensor("out", [12, B], F32, kind="ExternalOutput").ap()

    ENG3 = None  # set inside context

    with tile.TileContext(nc) as tc:
      with tc.tile_pool(name="pm", bufs=1) as pm, \
           tc.tile_pool(name="pd", bufs=1, space="DRAM") as pd:
        ENG3 = [nc.sync, nc.scalar, nc.gpsimd]

        def allred(tag, src, K):
            bi = pd.tile([128, K], F32, tag="ari" + tag, name="ari" + tag)
            bo = pd.tile([128 * NC, K], F32, tag="aro" + tag, name="aro" + tag)
            nc.sync.dma_start(bi[:], src)
            nc.gpsimd.collective_compute("AllGather", ALU.bypass, replica_groups=RG,
                                         ins=[bi.opt()], outs=[bo.opt()])
            gsb = pm.tile([128, K * NC], F32, tag="arg" + tag, name="arg" + tag)
            nc.sync.dma_start(gsb[:].rearrange("p (k r) -> p k r", r=NC),
                              bo[:].rearrange("(r p) k -> p k r", p=128))
            s = pm.tile([128, K], F32, tag="ars" + tag, name="ars" + tag)
            nc.vector.tensor_reduce(s[:], gsb[:].rearrange("p (k r) -> p k r", r=NC),
                                    axis=AX.X, op=ALU.add)
            return s

        # persistent weights
        w1s = pm.tile([128, 128], F32R)
        nc.gpsimd.memset(w1s[:].bitcast(F32), 0.0)
        nc.sync.dma_start(w1s[0:36, :], w1b)
        w2s = pm.tile([96, 192], F32R2);  nc.scalar.dma_start(w2s[:], w2)
        w3s = pm.tile([128, 768], BF16); nc.scalar.dma_start(w3s[:], w3p)
        wfcs = pm.tile([128, 6400], BF16, tag="wfcs", name="wfcs")
        nc.scalar.dma_start(wfcs[:], wfc)
        wfcos = pm.tile([128, 24], F32); nc.scalar.dma_start(wfcos[:], wfco)
        fcobs = pm.tile([12, 1], F32);   nc.scalar.dma_start(fcobs[:], fcob)
        ff4s = pm.tile([128, 32], F32);  nc.scalar.dma_start(ff4s[:], ff4)
        fb4s = pm.tile([32, 128], F32);  nc.scalar.dma_start(fb4s[:], fb4)
        ff2s = pm.tile([128, 64], F32);  nc.scalar.dma_start(ff2s[:], ff2)
        fb2s = pm.tile([64, 128], F32);  nc.scalar.dma_start(fb2s[:], fb2)

        # cross-stage tiles
        p2b = pm.tile([128, P2W], F32, tag="p2b", name="p2b")
        sump = pm.tile([128, 160], F32, tag="sump", name="sump")
        sqp = pm.tile([128, 160], F32, tag="sqp", name="sqp")
        ones = pm.tile([128, 320], F32, tag="ones", name="ones")
        nc.vector.memset(ones[:], 1.0)
        m2p = pm.tile([128, 16], F32, tag="m2p", name="m2p")
        m3p = pm.tile([128, 32], F32, tag="m3p", name="m3p")

        # ============ stage 1: conv1 -> stats + maxpool (42-padded out) ============
        with tc.tile_pool(name="pA", bufs=1) as pA:
          pooled = [pA.tile([128, 1764], F32, tag=f"pr{g}", name=f"pr{g}")
                    for g in range(8)]
          for g in range(8):
              nc.gpsimd.memset(pooled[g][:], 0.0)
          with tc.tile_pool(name="p1", bufs=2) as p1, \
               tc.tile_pool(name="pp1", bufs=4, space="PSUM") as pp1:
            for g in range(8):
                im = p1.tile([128, 6400], F32R, tag="im", name="im")
                nc.sync.dma_start(im[:], xim[g])
                for k in range(20):
                    ps = pp1.tile([128, 320], F32, tag="ps", name="c1ps")
                    nc.tensor.matmul(ps[:], w1s[:], im[:, 320 * k:320 * k + 320],
                                     start=True, stop=True)
                    idx = g * 20 + k
                    # evacuate PSUM once on Vector, folding in the sum for BN1
                    hb = p1.tile([128, 320], F32, tag="hb", name="hb")
                    nc.vector.scalar_tensor_tensor(hb[:], ps[:], 1.0, ones[:],
                                                   op0=ALU.mult, op1=ALU.mult,
                                                   accum_out=sump[:, idx:idx + 1])
                    # 2x2 maxpool: two pairwise-max steps from SBUF
                    px = p1.tile([128, 160], F32, tag="px", name="px")
                    w_ = hb[:].rearrange("p (y x two) -> p y x two", y=4, two=2)
                    nc.vector.tensor_tensor(px[:].rearrange("p (y x) -> p y x", y=4),
                                            w_[:, :, :, 0], w_[:, :, :, 1], op=ALU.max)
                    v_ = px[:].rearrange("p (y2 y x) -> p y2 y x", y2=2, y=2)
                    nc.vector.tensor_tensor(
                        pooled[g][:, 43 + 84 * k:43 + 84 * k + 84]
                            .rearrange("p (y x) -> p y x", x=42)[:, 0:2, 0:40],
                        v_[:, :, 0, :], v_[:, :, 1, :], op=ALU.max)
                    # square-sum on Scalar from SBUF (in-place square into hb)
                    nc.scalar.activation(hb[:], hb[:], ACTF.Square,
                                         accum_out=sqp[:, idx:idx + 1])

          # ---- BN1 stats: reduce partials, all-reduce, fold bands, broadcast ----
          st1 = pm.tile([128, 2], F32, tag="st1", name="st1")
          nc.vector.tensor_reduce(st1[:, 0:1], sump[:], axis=AX.X, op=ALU.add)
          nc.vector.tensor_reduce(st1[:, 1:2], sqp[:], axis=AX.X, op=ALU.add)
          sg1 = allred("1", st1[:], 2)
          f1s = pm.tile([32, 2], F32, tag="f1s", name="f1s")
          with tc.tile_pool(name="ppx1", bufs=1, space="PSUM") as ppx:
            psf = ppx.tile([32, 2], F32, tag="psf", name="psf")
            nc.tensor.matmul(psf[:], ff4s[:], sg1[:], start=True, stop=True)
            nc.scalar.copy(f1s[:], psf[:])
          m1 = pm.tile([32, 1], F32, tag="m1", name="m1")
          e2 = pm.tile([32, 1], F32, tag="e2", name="e2")
          nc.scalar.mul(m1[:], f1s[:, 0:1], 1.0 / N1)
          nc.scalar.mul(e2[:], f1s[:, 1:2], 1.0 / N1)
          v1 = pm.tile([32, 1], F32, tag="v1", name="v1")
          nc.vector.tensor_tensor(v1[:], m1[:], m1[:], op=ALU.mult)
          nc.vector.tensor_tensor(v1[:], e2[:], v1[:], op=ALU.subtract)
          nc.vector.tensor_scalar_add(v1[:], v1[:], EPS)
          sd = pm.tile([32, 1], F32, tag="sd", name="sd")
          nc.scalar.activation(sd[:], v1[:], ACTF.Sqrt, bias=0.0)
          rsb = pm.tile([32, 3], F32, tag="rsb", name="rsb")
          nc.vector.reciprocal(rsb[:, 0:1], sd[:])
          nc.vector.tensor_tensor(rsb[:, 1:2], m1[:], rsb[:, 0:1], op=ALU.mult)
          nc.scalar.mul(rsb[:, 1:2], rsb[:, 1:2], -1.0)
          nc.scalar.mul(rsb[:, 2:3], m1[:], -1.0)
          sc1b = pm.tile([128, 3], F32, tag="sc1b", name="sc1b")
          with tc.tile_pool(name="ppx2", bufs=1, space="PSUM") as ppx:
            psb = ppx.tile([128, 3], F32, tag="psb", name="psb")
            nc.tensor.matmul(psb[:], fb4s[:], rsb[:], start=True, stop=True)
            nc.scalar.copy(sc1b[:], psb[:])
          if dbg:
              nc.sync.dma_start(dout("d_sc1b", [128, 2]), sc1b[:])
              nc.sync.dma_start(dout("d_st1", [128, 2]), st1[:])

          # ---- BN1 apply (interior only, keeps halos zero) ----
          for g in range(8):
            vw = pooled[g][:, 43:43 + 1680].rearrange("p (y x) -> p y x", x=42)[:, :, 0:40]
            nc.scalar.activation(vw, vw, ACTF.Relu,
                                 bias=sc1b[:, 1:2], scale=sc1b[:, 0:1])
          if dbg:
              nc.sync.dma_start(dout("d_pool0", [128, 1764]), pooled[0][:])

          # ============ stage 2: conv2 -> mean + maxpool (22-padded out) ============
          with tc.tile_pool(name="p2", bufs=2) as p2, \
               tc.tile_pool(name="pp2", bufs=2, space="PSUM") as pp2:
            for g in range(8):
              a1c = p2.tile([96, A1W], F32R2, tag="a1c", name="a1c")
              if g < 2:
                  nc.gpsimd.memset(a1c[:].bitcast(F32), 0.0)
              for dx in range(3):
                for l in range(4):
                  ENG3[(dx * 4 + l) % 3].dma_start(
                      a1c[32 * dx:32 * dx + 32,
                          l * 1764 + 43 - dx:l * 1764 + 43 - dx + 1764],
                      pooled[g][32 * l:32 * l + 32, :].bitcast(F32R2))
              if dbg and g == 0:
                  nc.sync.dma_start(dout("d_a1c", [96, A1W]), a1c[:])
              for p in range(2):
                gp = g * 2 + p
                pc = pp2.tile([128, 2048], F32, tag="pcB", name="c2ps")
                for ck in range(4):
                    co, w = ck * 512, (512 if ck < 3 else 144)
                    for dyi in range(3):
                        for t2 in range(2):
                            base = 84 + (2 * p + t2) * 1764 + co + 42 * (dyi - 1)
                            nc.tensor.matmul(pc[64 * t2:64 * t2 + 64, co:co + w],
                                w2s[:, dyi * 64:dyi * 64 + 64],
                                a1c[:, base:base + w],
                                start=(dyi == 0), stop=(dyi == 2),
                                tile_position=(0, 64 * t2))
                vv = pc[:, 0:1680].rearrange("p (y x) -> p y x", x=42)[:, :, 1:41]
                nc.vector.tensor_reduce(m2p[:, gp:gp + 1], vv, axis=AX.XY, op=ALU.add)
                nc.vector.tensor_reduce(
                    p2b[:, gp * 484 + 23:gp * 484 + 23 + 440]
                        .rearrange("p (y x) -> p y x", x=22)[:, :, 0:20],
                    vv.rearrange("p (y2 y) (x2 x) -> p y2 x2 y x", y=2, x=2),
                    axis=AX.XY, op=ALU.max)

        # ---- BN2 mean: all-reduce, fold 2 bands, broadcast ----
        l2c = pm.tile([128, 1], F32, tag="l2c", name="l2c")
        nc.vector.tensor_reduce(l2c[:], m2p[:], axis=AX.X, op=ALU.add)
        sg2 = allred("2", l2c[:], 1)
        bi2b = pm.tile([128, 1], F32, tag="bi2b", name="bi2b")
        with tc.tile_pool(name="ppx3", bufs=1, space="PSUM") as ppx:
            psf = ppx.tile([64, 1], F32, tag="ps2f", name="ps2f")
            nc.tensor.matmul(psf[:], ff2s[:], sg2[:], start=True, stop=True)
            f2s = pm.tile([64, 1], F32, tag="f2s", name="f2s")
            nc.scalar.mul(f2s[:], psf[:], -1.0 / 409600.0)
            psb = ppx.tile([128, 1], F32, tag="ps2b", name="ps2b")
            nc.tensor.matmul(psb[:], fb2s[:], f2s[:], start=True, stop=True)
            nc.scalar.copy(bi2b[:], psb[:])
        if dbg:
            nc.sync.dma_start(dout("d_bi2b", [128, 1]), bi2b[:])
            nc.sync.dma_start(dout("d_p2b", [128, P2W]), p2b[:])

        # ============ stage 3: sign2 -> conv3 -> mean + sign-pool ============
        p3b = pm.tile([128, 3200], F32, tag="p3b", name="p3b")
        with tc.tile_pool(name="p3", bufs=2) as p3pool, \
             tc.tile_pool(name="pp3", bufs=4, space="PSUM") as pp3:
          b2p = p3pool.tile([128, B2W], BF16, tag="b2p", name="b2p", bufs=1)
          nc.gpsimd.memset(b2p[:], 0.0)
          for i in range(B):
            gp, t2 = i // 2, i % 2
            src = p2b[64 * t2:64 * t2 + 64, gp * 484 + 23:gp * 484 + 23 + 440] \
                .rearrange("p (y x) -> p y x", x=22)[:, :, 0:20]
            dst = b2p[0:64, 24 + i * 484 + 23: 24 + i * 484 + 23 + 440] \
                .rearrange("p (y x) -> p y x", x=22)[:, :, 0:20]
            nc.scalar.activation(dst, src, ACTF.Sign,
                                 bias=bi2b[64 * t2:64 * t2 + 64, :])
          for i in range(B):
            c0 = 24 + i * 484 - 2
            ENG3[i % 2].dma_start(b2p[64:128, c0:c0 + 444], b2p[0:64, c0 + 22:c0 + 466])
          for i in range(B):
            p3 = pp3.tile([128, 440], F32, tag="p3ps", name="c3ps")
            for dx in range(3):
                bp = 24 + i * 484 + (dx - 1)
                nc.tensor.matmul(p3[:], w3s[:, dx * 128:(dx + 1) * 128],
                                 b2p[:, bp:bp + 440],
                                 start=(dx == 0), stop=False)
            for dx in range(3):
                bs = 24 + i * 484 + 44 + (dx - 1)
                nc.tensor.matmul(p3[:], w3s[0:64, 384 + dx * 128:384 + (dx + 1) * 128],
                                 b2p[0:64, bs:bs + 440],
                                 start=False, stop=(dx == 2))
            vv = p3[:, 0:440].rearrange("p (y x) -> p y x", x=22)[:, :, 1:21]
            nc.vector.tensor_reduce(m3p[:, i:i + 1], vv, axis=AX.XY, op=ALU.add)
            nc.vector.tensor_reduce(
                p3b[:, i * 100:(i + 1) * 100].rearrange("p (y x) -> p y x", y=10),
                vv.rearrange("p (y2 y) (x2 x) -> p y2 x2 y x", y=2, x=2),
                axis=AX.XY, op=ALU.max)

        l3c = pm.tile([128, 1], F32, tag="l3c", name="l3c")
        nc.vector.tensor_reduce(l3c[:], m3p[:], axis=AX.X, op=ALU.add)
        sg3 = allred("3", l3c[:], 1)
        nc.scalar.mul(sg3[:], sg3[:], -1.0 / 102400.0)
        if dbg:
            nc.sync.dma_start(dout("d_sg3", [128, 1]), sg3[:])
            nc.sync.dma_start(dout("d_p3b", [128, 3200]), p3b[:])

        # ============ stage 4: sign3, avgpool, fc1, bn1d sign, fco ============
        with tc.tile_pool(name="p4", bufs=1) as p4, \
             tc.tile_pool(name="pp4", bufs=1, space="PSUM") as pp4:
          s3t = p4.tile([128, 3200], BF16, tag="s3t", name="s3t")
          nc.scalar.activation(s3t[:], p3b[:], ACTF.Sign, bias=sg3[:])
          zx = p4.tile([128, 1600], BF16, tag="zx", name="zx")
          v = s3t[:].rearrange("p (a x) -> p a x", x=2)
          nc.vector.tensor_tensor(zx[:], v[:, :, 0], v[:, :, 1], op=ALU.add)
          z2 = p4.tile([128, 800], BF16, tag="z2", name="z2")
          u = zx[:].rearrange("p (i y2 y x) -> p i y2 y x", i=32, y2=5, y=2)
          nc.vector.tensor_tensor(z2[:].rearrange("p (i y x) -> p i y x", i=32, y=5),
                                  u[:, :, :, 0, :], u[:, :, :, 1, :], op=ALU.add)
          zr = z2[:].rearrange("p (i s) -> p s i", s=25)
          mfp = pm.tile([128, 2], F32, tag="mfp", name="mfp")
          zss = []
          for hh in range(2):
              zs = pp4.tile([128, 32], F32, tag=f"zs{hh}", name=f"fcps{hh}")
              for sp in range(25):
                  nc.tensor.matmul(zs[:], wfcs[:, sp * 256 + 128 * hh: sp * 256 + 128 * hh + 128],
                                   zr[:, sp, :], start=(sp == 0), stop=(sp == 24))
              nc.vector.tensor_reduce(mfp[:, hh:hh + 1], zs[:], axis=AX.X, op=ALU.add)
              zss.append(zs)
          g4 = allred("4", mfp[:], 2)
          nc.scalar.mul(g4[:], g4[:], -1.0 / 256.0)
          ssb = pm.tile([128, 64], F32, tag="ssb", name="ssb")
          for hh in range(2):
              nc.scalar.activation(ssb[:, 32 * hh:32 * hh + 32], zss[hh][:],
                                   ACTF.Sign, bias=g4[:, hh:hh + 1])
          po = pp4.tile([12, 32], F32, tag="po", name="fops")
          for hh in range(2):
              nc.tensor.matmul(po[:], wfcos[:, hh * 12:hh * 12 + 12],
                               ssb[:, 32 * hh:32 * hh + 32],
                               start=(hh == 0), stop=(hh == 1))
          osb = pm.tile([12, 32], F32, tag="osb", name="osb")
          nc.scalar.activation(osb[:], po[:], ACTF.Identity, bias=fcobs[:], scale=1.0)
          nc.sync.dma_start(out, osb[:])

    nc.compile()
    return nc


def _prep_host(inputs):
    x = np.asarray(inputs["x"], np.float32)
    w1 = np.asarray(inputs["conv1_w"], np.float32)
    w2 = np.sign(np.asarray(inputs["w2"], np.float32))
    w3 = np.sign(np.asarray(inputs["w3"], np.float32))
    fc1 = np.sign(np.asarray(inputs["fc1_w"], np.float32))
    fco_w = np.asarray(inputs["fco_w"], np.float32)
    fco_b = np.asarray(inputs["fco_b"], np.float32)

    w1b = np.zeros((36, 128), np.float32)
    for r in range(4):
        w1b[9 * r:9 * r + 9, 32 * r:32 * r + 32] = w1[:, 0].reshape(32, 9).T
    w2m = np.zeros((96, 192), np.float32)
    for dyi in range(3):
        for dxi in range(3):
            w2m[32 * dxi:32 * dxi + 32, dyi * 64:(dyi + 1) * 64] = w2[:, :, dyi, dxi].T
    w3m = np.zeros((128, 768), ml_dtypes.bfloat16)
    for dx in range(3):
        w3m[0:64, dx * 128:(dx + 1) * 128] = w3[:, :, 0, dx].T.astype(ml_dtypes.bfloat16)
        w3m[64:128, dx * 128:(dx + 1) * 128] = w3[:, :, 1, dx].T.astype(ml_dtypes.bfloat16)
        w3m[0:64, 384 + dx * 128:384 + (dx + 1) * 128] = w3[:, :, 2, dx].T.astype(ml_dtypes.bfloat16)
    wfcm = np.zeros((128, 6400), ml_dtypes.bfloat16)
    fc1r = fc1.reshape(256, 128, 25)
    for sp in range(25):
        wfcm[:, sp * 256:(sp + 1) * 256] = fc1r[:, :, sp].T.astype(ml_dtypes.bfloat16)
    wfcom = np.zeros((128, 24), np.float32)
    wfcom[:, 0:12] = fco_w[:, 0:128].T
    wfcom[:, 12:24] = fco_w[:, 128:256].T
    fcobm = fco_b.reshape(12, 1).astype(np.float32)

    ff4 = np.zeros((128, 32), np.float32)
    for r in range(4):
        ff4[32 * r:32 * r + 32, :] = np.eye(32, dtype=np.float32)
    fb4 = ff4.T.copy()
    ff2 = np.zeros((128, 64), np.float32)
    for r in range(2):
        ff2[64 * r:64 * r + 64, :] = np.eye(64, dtype=np.float32)
    fb2 = ff2.T.copy()

    in_maps = []
    for core in range(NC):
        xs = x[core * B:(core + 1) * B, 0]
        xpad = np.zeros((B, 82, 82), np.float32)
        xpad[:, 1:81, 1:81] = xs
        win = np.lib.stride_tricks.sliding_window_view(xpad, (80, 80), axis=(1, 2))
        xim = np.zeros((8, 128, 6400), np.float32)
        xim[:, 0:36, :] = win.reshape(8, 36, 6400)
        in_maps.append({"xim": xim, "w1b": w1b, "w2": w2m, "w3p": w3m,
                        "wfc": wfcm, "wfco": wfcom, "fcob": fcobm,
                        "ff4": ff4, "fb4": fb4, "ff2": ff2, "fb2": fb2})
    return in_maps


def kernel(**inputs):
    dbg = bool(int(os.environ.get("BNN_DEBUG", "0")))
    if "nc" not in _cache:
        _cache["nc"] = _build(dbg=dbg)
    nc = _cache["nc"]
    in_maps = _prep_host(inputs)
    trace = bool(int(os.environ.get("BNN_TRACE", "0")))
    if trace:
        sys.path.insert(0, os.path.dirname(os.path.abspath(__file__)))
        try:
            import ntff_shim
            ntff_shim.install()
        except Exception:
            pass
    tdir = os.environ.get("BNN_TRACE_DIR") if trace else None
    res = bass_utils.run_bass_kernel_spmd(nc, in_maps, core_ids=list(range(NC)), trace=trace,
                                          tmpdir=tdir)
    _cache["exec_time_ns"] = res.exec_time_ns
    if trace and res.instructions_and_trace:
        _cache["trace_path"] = res.instructions_and_trace[1]
    _cache["results"] = res.results
    out = np.zeros((256, 12), np.float32)
    for core in range(NC):
        out[core * B:(core + 1) * B, :] = res.results[core]["out"].T
    return out
